# revision 7
# baseline (speedup 1.0000x reference)
"""CKAN (gnn_message_passing) Trainium2 kernel, v3.

Data-parallel over 8 NeuronCores (512 batch rows each), no collectives.
Entity rows are fetched in two levels to fit dma_gather's int16 index
limit; the compact unique table lives in SBUF:
  phase A: sorted-unique h∪t rows gathered from a host-padded f16 table
           (row = [e(64 f16) | pad]) with windowed calls directly into a
           [128, na/128, 128] SBUF tile (row i -> partition i%128,
           slot i//128);
  phase B: per-(b,t) rows fetched with SBUF-source transpose-gathers,
           which deliver rows FEATURE-major: h rows land as the MLP's
           rhs directly, t rows are transposed back to batch-major on
           the PE per 128-column chunk.
The relation contribution enters the layer-1 matmul as a host-shipped
one-hot [32, n_col] DMA'd into partitions 64:96 of the h region of the
gather-output tile, so layer 1 is a single [96->128] matmul against
w1r = [[gate_w1h | att_w1h], rel_emb @ [gate_w1d | att_w1d]].
gate2 and att3 run as "flip" matmuls producing batch-major [g2 | s3].
Softmax over the 32 neighbors is computed in 4 quarter-partials that
are renormalized and combined per stage.

v3 restructure for pipeline overlap (v2 ran phaseA -> phaseB -> compute
nearly serially; the SWDGE queues sat in ~60us dependency stalls):
each tower-layer is split into TWO half-towers of 256 batch rows (8
stages total, smaller tables, table pool bufs=2), and stage s+1's
phase-A gathers are emitted BEFORE stage s's body, so A(s+1) DMA
drains under compute(s) and B(s) never waits on a fresh table.
"""
import sys
sys.path.insert(0, '/opt/trn_rl_repo')
import numpy as np

# ---- problem dims (overridable for small-scale sim tests) ----
DIM = 64
N_ENTITY = 100000
N_RELATION = 32
N_LAYER = 2
B = 4096
T = 32
N_CORES = 8
WIN = 32768
NW_CAP = None   # computed in _dims()
_NC_CACHE = None

NQ = 4            # softmax quarter-partials
NS = 8            # pipeline stages (tower-layer halves)


def _dims():
    b_core = B // N_CORES
    b_stage = b_core // 2
    n_col = b_stage * T            # columns per stage
    nwin = (N_ENTITY + WIN - 1) // WIN
    if NW_CAP is not None:
        caps = list(NW_CAP)
    else:
        # expected uniques per full window for 2*n_col draws over N_ENTITY,
        # with ~8 sigma headroom, rounded to 128
        import math
        caps = []
        for w in range(nwin):
            width = min(WIN, N_ENTITY - WIN * w)
            mean = width * (1.0 - (1.0 - 1.0 / N_ENTITY) ** (2 * n_col))
            cap = int(mean + 8 * math.sqrt(max(mean, 1.0)) + 64)
            caps.append(-(-cap // 128) * 128)
    na = sum(caps)
    assert na % 128 == 0 and na <= 32767
    return b_core, b_stage, n_col, nwin, caps, na


def _wrap_idx16(a):
    """int16 vector -> dma_gather idx layout [128, ceil(n/16)]."""
    a = np.asarray(a, dtype=np.int16)
    n = len(a)
    pad = (-n) % 16
    if pad:
        a = np.concatenate([a, np.full(pad, -1, np.int16)])
    w = a.reshape(-1, 16).T.copy()
    return np.tile(w, (8, 1))


def _host_prep_tl(h_flat, t_flat):
    b_core, b_stage, n_col, nwin, caps, na = _dims()
    uni = np.unique(np.concatenate([h_flat, t_flat]))
    val_to_pos = np.full(N_ENTITY, -1, np.int32)
    idxA_parts = []
    off = 0
    for w in range(nwin):
        lo, hi = WIN * w, min(WIN * (w + 1), N_ENTITY)
        seg = uni[(uni >= lo) & (uni < hi)]
        cap = caps[w]
        assert len(seg) <= cap, f"window {w} overflow: {len(seg)} > {cap}"
        val_to_pos[seg] = off + np.arange(len(seg), dtype=np.int32)
        assert len(seg) == 0 or off + len(seg) - 1 <= 32767, "position overflow"
        fill = (seg[-1] - lo) if len(seg) else 0
        seg_l = np.concatenate([(seg - lo).astype(np.int16),
                                np.full(cap - len(seg), fill, np.int16)])
        idxA_parts.append(seg_l)
        off += cap
    idxA = np.concatenate(idxA_parts)
    h_loc = val_to_pos[h_flat]
    t_loc = val_to_pos[t_flat]
    assert (h_loc >= 0).all() and (t_loc >= 0).all()
    return idxA, h_loc.astype(np.int16), t_loc.astype(np.int16)


TL_LIST = [("u", 0), ("u", 1), ("i", 0), ("i", 1)]


def _build_program(debug=True):
    import concourse.bacc as bacc
    import concourse.tile as tile
    from concourse import mybir
    from concourse.masks import make_identity
    from concourse import tile_sem_assignment as tsa

    # Tile assigns DMASW sem lanes round-robin in scheduled order, but each
    # lane is hardware-locked to SWDGE queue (lane % 4).  Force gather
    # instructions onto lanes consistent with their queue_num.
    if not getattr(tsa.TileClockTick, "_gather_lane_patched", False):
        _orig_assign_tick = tsa.TileClockTick._assign_tick

        def _patched_assign_tick(self, inst):
            if isinstance(inst, mybir.InstDMAGatherAnt):
                q = inst.queue_num
                tog = getattr(self, "_gather_lane_toggle", None)
                if tog is None:
                    tog = self._gather_lane_toggle = {}
                k = tog.get(q, 0)
                tog[q] = k ^ 1
                saved = self.next_sw_dma_idx
                self.next_sw_dma_idx = q + 4 * k
                try:
                    return _orig_assign_tick(self, inst)
                finally:
                    self.next_sw_dma_idx = saved
            return _orig_assign_tick(self, inst)

        tsa.TileClockTick._assign_tick = _patched_assign_tick
        tsa.TileClockTick._gather_lane_patched = True

    f32 = mybir.dt.float32
    f16 = mybir.dt.float16
    i16 = mybir.dt.int16
    AF = mybir.ActivationFunctionType
    ALU = mybir.AluOpType
    AX = mybir.AxisListType

    b_core, b_stage, n_col, nwin, caps, na = _dims()
    NB = b_stage // 128              # b-chunks per t (2 at full scale)
    NBF = b_core // 128              # full-core b-chunks (4)
    TQ = T // NQ                     # t per quarter (8)
    TPT = 2                          # t's per MLP tile
    TILE = TPT * b_stage             # columns per MLP tile (512)
    CPT = TPT * NB                   # 128-col chunks per tile (4)
    COLS_Q = b_stage * TQ            # columns per quarter (2048)
    TLQ = COLS_Q // TILE             # MLP tiles per quarter (4)
    na_slots = na // 128
    GMAX = 4096                      # max idx per gather call

    nc = bacc.Bacc("TRN2", target_bir_lowering=False, debug=debug,
                   num_swdge_queues=4)
    _qctr = [0]

    def _nextq():
        q = _qctr[0] % 4
        _qctr[0] += 1
        return q

    entp = nc.dram_tensor("entp", [N_ENTITY, 2 * DIM], f16, kind="ExternalInput")
    w1r = nc.dram_tensor("w1r", [DIM + N_RELATION, 128], f16, kind="ExternalInput")
    w2 = nc.dram_tensor("w2", [64, 64], f16, kind="ExternalInput")
    wflip = nc.dram_tensor("wflip", [128, 65], f16, kind="ExternalInput")
    items16 = nc.dram_tensor("items16", [128, max(b_core // 16, 1)], i16,
                             kind="ExternalInput")
    idxA, idxQ, ohr = {}, {}, {}
    for s in range(NS):
        idxA[s] = nc.dram_tensor(f"idxA{s}", [128, na // 16], i16,
                                 kind="ExternalInput")
        idxQ[s] = nc.dram_tensor(f"idxQ{s}", [128, 2 * n_col // 16], i16,
                                 kind="ExternalInput")
        ohr[s] = nc.dram_tensor(f"ohr{s}", [N_RELATION, n_col], f16,
                                kind="ExternalInput")
    scores_hbm = nc.dram_tensor("scores", [b_core], f32, kind="ExternalOutput")

    # flat list of (window, offset, count) phase-A calls, <=GMAX idx
    acalls = []
    off = 0
    for w in range(nwin):
        nw_ = caps[w]
        nsplit = max(1, -(-nw_ // GMAX))
        step = -(-(-(-nw_ // nsplit)) // 128) * 128
        done = 0
        while done < nw_:
            nn = min(step, nw_ - done)
            acalls.append((w, off + done, nn))
            done += nn
        off += nw_

    with tile.TileContext(nc) as tc:
        with (
            tc.tile_pool(name="const", bufs=1) as cp,
            tc.tile_pool(name="tab", bufs=2) as tabp,
            tc.tile_pool(name="xt", bufs=2) as xtp,
            tc.tile_pool(name="te", bufs=2) as tep,
            tc.tile_pool(name="gsq", bufs=2) as gsp,
            tc.tile_pool(name="work", bufs=2) as wp,
            tc.tile_pool(name="idx", bufs=3) as ixp,
            tc.tile_pool(name="keep", bufs=1) as kp,
            tc.tile_pool(name="ps1", bufs=2, space="PSUM") as ps1,
            tc.tile_pool(name="ps2", bufs=2, space="PSUM") as ps2,
            tc.tile_pool(name="psF", bufs=2, space="PSUM") as psF,
            tc.tile_pool(name="psT", bufs=2, space="PSUM") as psT,
        ):
            ident = cp.tile([128, 128], f32)
            make_identity(nc, ident[:])
            ident16 = cp.tile([128, 128], f16)
            nc.vector.tensor_copy(out=ident16[:], in_=ident[:])
            w1r_sb = cp.tile([DIM + N_RELATION, 128], f16)
            nc.sync.dma_start(out=w1r_sb[:], in_=w1r[:])
            w2_sb = cp.tile([64, 64], f16)
            nc.sync.dma_start(out=w2_sb[:], in_=w2[:])
            wflip_sb = cp.tile([128, 65], f16)
            nc.sync.dma_start(out=wflip_sb[:], in_=wflip[:])

            items_sb = cp.tile([128, max(b_core // 16, 1)], i16)
            nc.sync.dma_start(out=items_sb[:], in_=items16[:])
            iorig = kp.tile([128, NBF, 2 * DIM], f16)
            nc.gpsimd.dma_gather(
                out_ap=iorig[:], in_ap=entp[:], idxs_ap=items_sb[:],
                num_idxs=b_core, num_idxs_reg=b_core, elem_size=2 * DIM,
                queue_num=_nextq(), single_packet=False)

            otl = {}                 # k -> [128, NBF, DIM] f32 layer outputs
            uorig_bm = kp.tile([128, NBF, DIM], f32)
            tables = [None] * NS
            iqs = [None] * NS

            def emit_A(s):
                ia = ixp.tile([128, na // 16], i16, name=f"ia{s}", tag="ia")
                nc.sync.dma_start(out=ia[:], in_=idxA[s][:])
                iq = ixp.tile([128, 2 * n_col // 16], i16, name=f"iq{s}",
                              tag="iq")
                nc.sync.dma_start(out=iq[:], in_=idxQ[s][:])
                iqs[s] = iq
                table = tabp.tile([128, na_slots, 2 * DIM], f16,
                                  name=f"table{s}", tag="table")
                tables[s] = table
                for (w, o2, nn) in acalls:
                    nc.gpsimd.dma_gather(
                        out_ap=table[:, o2 // 128:(o2 + nn) // 128, :],
                        in_ap=entp[WIN * w: min(WIN * (w + 1), N_ENTITY), :],
                        idxs_ap=ia[:, o2 // 16:(o2 + nn) // 16],
                        num_idxs=nn, num_idxs_reg=nn, elem_size=2 * DIM,
                        queue_num=_nextq(), single_packet=False)

            def emit_body(s):
                k, half = s // 2, s % 2
                tw, l = TL_LIST[k]
                table, iq = tables[s], iqs[s]

                exh = kp.tile([128, NB, T], f16, name=f"exh{s}", tag="exh",
                              bufs=2)
                nmh = kp.tile([128, NB, NQ], f32, name=f"nmh{s}", tag="nmh",
                              bufs=2)
                esh = kp.tile([128, NB, NQ], f32, name=f"esh{s}", tag="esh",
                              bufs=2)
                psum_t = kp.tile([128, NB, NQ, DIM], f32, name=f"pst{s}",
                                 tag="pst", bufs=2)
                if tw == "u" and l == 0:
                    uacc = kp.tile([64, b_stage], f32, name=f"uacc{s}",
                                   tag="uacc", bufs=2)

                for qu in range(NQ):
                    xt = xtp.tile([128, 1, 2 * COLS_Q], f16,
                                  name=f"xt{s}_{qu}", tag="xt")
                    # one call: h rows -> cols [0, COLS_Q), t -> [COLS_Q, 2C)
                    iqo = qu * (2 * COLS_Q) // 16
                    nc.gpsimd.dma_gather(
                        out_ap=xt[:],
                        in_ap=table[:],
                        idxs_ap=iq[:, iqo:iqo + 2 * COLS_Q // 16],
                        num_idxs=2 * COLS_Q, num_idxs_reg=2 * COLS_Q,
                        elem_size=2 * DIM, transpose=True,
                        sbuf_tokens_per_rank=128,
                        sbuf_free_dim_per_rank=4 * DIM,
                        queue_num=_nextq(), single_packet=False)
                    # one-hot relations into partitions 64:96 of the h cols
                    nc.sync.dma_start(
                        out=xt[DIM:DIM + N_RELATION, 0, 0:COLS_Q],
                        in_=ohr[s][:, qu * COLS_Q:(qu + 1) * COLS_Q])

                    te_sb = tep.tile([128, TQ, NB, DIM], f16,
                                     name=f"te{s}_{qu}", tag="te")
                    gsq = gsp.tile([128, TQ, NB, 65], f16,
                                   name=f"gs{s}_{qu}", tag="gsq")
                    for ti in range(TLQ):
                        cols = slice(ti * TILE, (ti + 1) * TILE)
                        tt0 = ti * TPT
                        p1 = ps1.tile([128, TILE], f32, space="PSUM",
                                      name="p1", tag="p1")
                        nc.tensor.matmul(out=p1[:],
                                         lhsT=w1r_sb[:],
                                         rhs=xt[0:DIM + N_RELATION, 0, cols],
                                         start=True, stop=True)
                        # t rows back to batch-major via PE transpose
                        pt = psT.tile([128, CPT, DIM], f16, space="PSUM",
                                      name="pt", tag="pt")
                        for c in range(CPT):
                            c0 = COLS_Q + ti * TILE + c * 128
                            nc.tensor.transpose(
                                out=pt[:, c, :],
                                in_=xt[0:DIM, 0, c0:c0 + 128],
                                identity=ident16[0:DIM, 0:DIM])
                        nc.scalar.activation(
                            out=te_sb[:, tt0:tt0 + TPT, :, :].rearrange(
                                "p t q d -> p (t q) d"),
                            in_=pt[:], func=AF.Copy)
                        lflip = wp.tile([128, TILE], f16, name="lflip",
                                        tag="lflip")
                        nc.vector.tensor_scalar(
                            out=lflip[0:64, :], in0=p1[0:64, :], scalar1=0.0,
                            scalar2=None, op0=ALU.max)
                        r1s = wp.tile([64, TILE], f16, name="r1s", tag="r1s")
                        nc.scalar.activation(out=r1s[:], in_=p1[64:128, :],
                                             func=AF.Relu)
                        p2 = ps2.tile([64, TILE], f32, space="PSUM",
                                      name="p2", tag="p2")
                        nc.tensor.matmul(out=p2[:], lhsT=w2_sb[:], rhs=r1s[:],
                                         start=True, stop=True)
                        nc.vector.tensor_scalar(
                            out=lflip[64:128, :], in0=p2[:], scalar1=0.0,
                            scalar2=None, op0=ALU.max)
                        pf = psF.tile([128, CPT, 65], f32, space="PSUM",
                                      name="pf", tag="pf")
                        for c in range(CPT):
                            nc.tensor.matmul(out=pf[:, c, :],
                                             lhsT=lflip[:, c * 128:(c + 1) * 128],
                                             rhs=wflip_sb[:],
                                             start=True, stop=True)
                        nc.scalar.activation(
                            out=gsq[:, tt0:tt0 + TPT, :, :].rearrange(
                                "p t q d -> p (t q) d"),
                            in_=pf[:], func=AF.Sigmoid)

                    # ---- per-quarter softmax partials + weighted sums ----
                    nc.vector.tensor_reduce(
                        out=nmh[:, :, qu:qu + 1],
                        in_=gsq[:, :, :, 64].rearrange("p t q -> p q t"),
                        axis=AX.X, op=ALU.max, negate=True)
                    for q in range(NB):
                        nc.scalar.activation(
                            out=exh[:, q, qu * TQ:(qu + 1) * TQ],
                            in_=gsq[:, :, q, 64:65], func=AF.Exp,
                            bias=nmh[:, q, qu:qu + 1],
                            accum_out=esh[:, q, qu:qu + 1])
                        wmul = wp.tile([128, TQ, DIM], f16, name="wmul",
                                       tag="wmul")
                        nc.vector.tensor_tensor(
                            out=wmul[:], in0=te_sb[:, :, q, :],
                            in1=exh[:, q, qu * TQ:(qu + 1) * TQ, None]
                                .to_broadcast([128, TQ, DIM]),
                            op=ALU.mult)
                        pmul = wp.tile([128, TQ, DIM], f16, name="pmul",
                                       tag="pmul")
                        nc.vector.tensor_tensor(
                            out=pmul[:], in0=wmul[:],
                            in1=gsq[:, :, q, 0:64], op=ALU.mult)
                        nc.vector.tensor_reduce(
                            out=psum_t[:, q, qu, :],
                            in_=pmul[:].rearrange("p t d -> p d t"),
                            axis=AX.X, op=ALU.add)

                    if tw == "u" and l == 0:
                        ured = wp.tile([64, b_stage], f32, name="ured",
                                       tag="ured")
                        nc.vector.tensor_reduce(
                            out=ured[:],
                            in_=xt[0:DIM, 0, 0:COLS_Q]
                                .rearrange("p (t b) -> p b t", b=b_stage),
                            axis=AX.X, op=ALU.add)
                        if qu == 0:
                            nc.vector.tensor_copy(out=uacc[:], in_=ured[:])
                        else:
                            nc.vector.tensor_add(out=uacc[:], in0=uacc[:],
                                                 in1=ured[:])

                # ---- combine quarters with softmax renormalization ----
                nmall = wp.tile([128, NB, 1], f32, name="nmall", tag="nmall")
                nc.vector.tensor_reduce(out=nmall[:], in_=nmh[:],
                                        axis=AX.X, op=ALU.min)
                dif = wp.tile([128, NB, NQ], f32, name="dif", tag="dif")
                nc.vector.tensor_tensor(
                    out=dif[:], in0=nmall[:].to_broadcast([128, NB, NQ]),
                    in1=nmh[:], op=ALU.subtract)
                sc = wp.tile([128, NB, NQ], f32, name="sc", tag="sc")
                nc.scalar.activation(out=sc[:], in_=dif[:], func=AF.Exp)
                stmp = wp.tile([128, NB, NQ], f32, name="stmp", tag="stmp")
                nc.vector.tensor_tensor(out=stmp[:], in0=esh[:], in1=sc[:],
                                        op=ALU.mult)
                tot = wp.tile([128, NB, 1], f32, name="tot", tag="tot")
                nc.vector.tensor_reduce(out=tot[:], in_=stmp[:], axis=AX.X,
                                        op=ALU.add)
                rec = wp.tile([128, NB, 1], f32, name="rec", tag="rec")
                nc.vector.reciprocal(out=rec[:], in_=tot[:])
                pw = wp.tile([128, NB, NQ, DIM], f32, name="pw", tag="pw")
                nc.vector.tensor_tensor(
                    out=pw[:], in0=psum_t[:],
                    in1=sc[:, :, :, None].to_broadcast([128, NB, NQ, DIM]),
                    op=ALU.mult)
                osum = wp.tile([128, NB, DIM], f32, name="osum", tag="osum")
                nc.vector.tensor_reduce(
                    out=osum[:], in_=pw[:].rearrange("p q h d -> p q d h"),
                    axis=AX.X, op=ALU.add)
                if k not in otl:
                    otl[k] = kp.tile([128, NBF, DIM], f32, name=f"otl{k}",
                                     tag=f"otl{k}")
                nc.vector.tensor_tensor(
                    out=otl[k][:, half * NB:(half + 1) * NB, :], in0=osum[:],
                    in1=rec[:].to_broadcast([128, NB, DIM]), op=ALU.mult)

                if tw == "u" and l == 0:
                    put = psT.tile([128, NB, DIM], f32, space="PSUM",
                                   name="put", tag="pt")
                    for q in range(NB):
                        nc.tensor.transpose(
                            out=put[:, q, :],
                            in_=uacc[:, q * 128:(q + 1) * 128],
                            identity=ident[0:64, 0:64])
                    nc.scalar.activation(
                        out=uorig_bm[:, half * NB:(half + 1) * NB, :],
                        in_=put[:], func=AF.Copy)

            # ---- software-pipelined emission ----
            for s in range(NS + 1):
                if s < NS:
                    emit_A(s)
                if s >= 1:
                    emit_body(s - 1)

            # ---- scores (2.0 gate scale twice, 1/T origin mean) ----
            m = wp.tile([128, NBF, DIM], f32, name="m", tag="m")
            nc.vector.tensor_tensor(out=m[:], in0=uorig_bm[:],
                                    in1=iorig[:, :, 0:DIM], op=ALU.mult)
            acc = wp.tile([128, NBF, DIM], f32, name="macc", tag="macc")
            nc.vector.tensor_scalar(out=acc[:], in0=m[:], scalar1=1.0 / T,
                                    scalar2=None, op0=ALU.mult)
            for ku, ki in ((0, 2), (1, 3)):
                mu = wp.tile([128, NBF, DIM], f32, name="mu", tag="mu")
                nc.vector.tensor_tensor(out=mu[:], in0=otl[ku][:],
                                        in1=otl[ki][:], op=ALU.mult)
                nc.vector.tensor_scalar(out=mu[:], in0=mu[:], scalar1=4.0,
                                        scalar2=None, op0=ALU.mult)
                nc.vector.tensor_add(out=acc[:], in0=acc[:], in1=mu[:])
            ssum = wp.tile([128, NBF, 1], f32, name="ssum", tag="ssum")
            nc.vector.tensor_reduce(out=ssum[:], in_=acc[:], axis=AX.X,
                                    op=ALU.add)
            sc_all = kp.tile([128, NBF], f32)
            nc.scalar.activation(out=sc_all[:], in_=ssum[:, :, 0],
                                 func=AF.Sigmoid)
            nc.sync.dma_start(out=scores_hbm.rearrange("(s p) -> p s", p=128),
                              in_=sc_all[:])
    nc.compile()
    return nc


def _make_in_maps(inputs):
    b_core, b_stage, n_col, nwin, caps, na = _dims()
    ent = np.asarray(inputs["ent_emb"], np.float32)
    rel = np.asarray(inputs["rel_emb"], np.float32)
    att_w1 = np.asarray(inputs["att_w1"], np.float32)
    att_w2 = np.asarray(inputs["att_w2"], np.float32)
    att_w3 = np.asarray(inputs["att_w3"], np.float32)
    gate_w1 = np.asarray(inputs["gate_w1"], np.float32)
    gate_w2 = np.asarray(inputs["gate_w2"], np.float32)
    items = np.asarray(inputs["items"]).astype(np.int64)
    idx6 = {n: np.asarray(inputs[n]).astype(np.int64)
            for n in ("user_h", "user_r", "user_t", "item_h", "item_r",
                      "item_t")}

    entp = np.zeros((N_ENTITY, 2 * DIM), np.float16)
    entp[:, 0:DIM] = ent.astype(np.float16)
    w1h = np.concatenate([gate_w1[:DIM], att_w1[:DIM]], axis=1)
    r1p = rel @ np.concatenate([gate_w1[DIM:], att_w1[DIM:]], axis=1)
    w1r = np.concatenate([w1h, r1p], axis=0).astype(np.float16)
    wflip = np.zeros((128, 65), np.float16)
    wflip[0:64, 0:64] = gate_w2.astype(np.float16)
    wflip[64:128, 64:65] = att_w3.astype(np.float16)

    in_maps = []
    for c in range(N_CORES):
        im = {
            "entp": entp, "w1r": w1r, "w2": att_w2.astype(np.float16),
            "wflip": wflip,
            "items16": _wrap_idx16(
                items[c * b_core:(c + 1) * b_core].astype(np.int16)),
        }
        for s in range(NS):
            k, half = s // 2, s % 2
            tw, l = TL_LIST[k]
            pre = "user" if tw == "u" else "item"
            lo = c * b_core + half * b_stage
            sl = slice(lo, lo + b_stage)
            h = idx6[f"{pre}_h"][l, sl].T.ravel()
            t = idx6[f"{pre}_t"][l, sl].T.ravel()
            r = idx6[f"{pre}_r"][l, sl].T.ravel()
            ia, hl, tl_ = _host_prep_tl(h, t)
            im[f"idxA{s}"] = _wrap_idx16(ia)
            # per-quarter [h_cols ++ t_cols] index stream
            cq = n_col // NQ
            parts = []
            for qu in range(NQ):
                parts.append(_wrap_idx16(hl[qu * cq:(qu + 1) * cq]))
                parts.append(_wrap_idx16(tl_[qu * cq:(qu + 1) * cq]))
            im[f"idxQ{s}"] = np.concatenate(parts, axis=1)
            oh = (r[None, :] == np.arange(N_RELATION)[:, None])
            im[f"ohr{s}"] = oh.astype(np.float16)
        in_maps.append(im)
    return in_maps


def kernel(**inputs):
    global _NC_CACHE
    import os
    from concourse.bass_utils import run_bass_kernel_spmd

    if _NC_CACHE is None:
        _NC_CACHE = _build_program()
    nc = _NC_CACHE
    in_maps = _make_in_maps(inputs)
    trace = bool(int(os.environ.get("CKAN_TRACE", "0")))
    res = run_bass_kernel_spmd(nc, in_maps, core_ids=list(range(N_CORES)),
                               trace=trace)
    if trace and res.exec_time_ns is not None:
        print(f"HW exec time: {res.exec_time_ns} ns")
    if trace and res.instructions_and_trace is not None:
        print(f"trace path: {res.instructions_and_trace[1]}")
    b_core = B // N_CORES
    out = np.concatenate([res.results[c]["scores"] for c in range(N_CORES)])
    return out.astype(np.float32)


# revision 9
# speedup vs baseline: 1.1800x; 1.1800x over previous
"""CKAN (gnn_message_passing) Trainium2 kernel, v3.

Data-parallel over 8 NeuronCores (512 batch rows each), no collectives.
Entity rows are fetched in two levels to fit dma_gather's int16 index
limit; the compact unique table lives in SBUF:
  phase A: sorted-unique h∪t rows gathered from a host-padded f16 table
           (row = [e(64 f16) | pad]) with windowed calls directly into a
           [128, na/128, 128] SBUF tile (row i -> partition i%128,
           slot i//128);
  phase B: per-(b,t) rows fetched with SBUF-source transpose-gathers,
           which deliver rows FEATURE-major: h rows land as the MLP's
           rhs directly, t rows are transposed back to batch-major on
           the PE per 128-column chunk.
The relation contribution enters the layer-1 matmul as a host-shipped
one-hot [32, n_col] DMA'd into partitions 64:96 of the h region of the
gather-output tile, so layer 1 is a single [96->128] matmul against
w1r = [[gate_w1h | att_w1h], rel_emb @ [gate_w1d | att_w1d]].
gate2 and att3 run as "flip" matmuls producing batch-major [g2 | s3].
Softmax over the 32 neighbors is computed in 4 quarter-partials that
are renormalized and combined per stage.

v3 restructure for pipeline overlap (v2 ran phaseA -> phaseB -> compute
nearly serially; the SWDGE queues sat in ~60us dependency stalls):
each tower-layer is split into TWO half-towers of 256 batch rows (8
stages total, smaller tables, table pool bufs=2), and stage s+1's
phase-A gathers are emitted BEFORE stage s's body, so A(s+1) DMA
drains under compute(s) and B(s) never waits on a fresh table.
"""
import sys
sys.path.insert(0, '/opt/trn_rl_repo')
import numpy as np

# ---- problem dims (overridable for small-scale sim tests) ----
DIM = 64
N_ENTITY = 100000
N_RELATION = 32
N_LAYER = 2
B = 4096
T = 32
N_CORES = 8
WIN = 32768
NW_CAP = None   # computed in _dims()
_NC_CACHE = None

NQ = 4            # softmax quarter-partials
NS = 8            # pipeline stages (tower-layer halves)


def _dims():
    b_core = B // N_CORES
    b_stage = b_core // 2
    n_col = b_stage * T            # columns per stage
    nwin = (N_ENTITY + WIN - 1) // WIN
    if NW_CAP is not None:
        caps = list(NW_CAP)
    else:
        # expected uniques per full window for 2*n_col draws over N_ENTITY,
        # with ~8 sigma headroom, rounded to 128
        import math
        caps = []
        for w in range(nwin):
            width = min(WIN, N_ENTITY - WIN * w)
            mean = width * (1.0 - (1.0 - 1.0 / N_ENTITY) ** (2 * n_col))
            cap = int(mean + 8 * math.sqrt(max(mean, 1.0)) + 64)
            caps.append(-(-cap // 128) * 128)
    na = sum(caps)
    assert na % 128 == 0 and na <= 32767
    return b_core, b_stage, n_col, nwin, caps, na


def _wrap_idx16(a):
    """int16 vector -> dma_gather idx layout [128, ceil(n/16)]."""
    a = np.asarray(a, dtype=np.int16)
    n = len(a)
    pad = (-n) % 16
    if pad:
        a = np.concatenate([a, np.full(pad, -1, np.int16)])
    w = a.reshape(-1, 16).T.copy()
    return np.tile(w, (8, 1))


def _host_prep_tl(h_flat, t_flat):
    b_core, b_stage, n_col, nwin, caps, na = _dims()
    uni = np.unique(np.concatenate([h_flat, t_flat]))
    val_to_pos = np.full(N_ENTITY, -1, np.int32)
    idxA_parts = []
    off = 0
    for w in range(nwin):
        lo, hi = WIN * w, min(WIN * (w + 1), N_ENTITY)
        seg = uni[(uni >= lo) & (uni < hi)]
        cap = caps[w]
        assert len(seg) <= cap, f"window {w} overflow: {len(seg)} > {cap}"
        val_to_pos[seg] = off + np.arange(len(seg), dtype=np.int32)
        assert len(seg) == 0 or off + len(seg) - 1 <= 32767, "position overflow"
        fill = (seg[-1] - lo) if len(seg) else 0
        seg_l = np.concatenate([(seg - lo).astype(np.int16),
                                np.full(cap - len(seg), fill, np.int16)])
        idxA_parts.append(seg_l)
        off += cap
    idxA = np.concatenate(idxA_parts)
    h_loc = val_to_pos[h_flat]
    t_loc = val_to_pos[t_flat]
    assert (h_loc >= 0).all() and (t_loc >= 0).all()
    return idxA, h_loc.astype(np.int16), t_loc.astype(np.int16)


TL_LIST = [("u", 0), ("u", 1), ("i", 0), ("i", 1)]


def _build_program(debug=True):
    import concourse.bacc as bacc
    import concourse.tile as tile
    from concourse import mybir
    from concourse.masks import make_identity
    from concourse import tile_sem_assignment as tsa

    # Tile assigns DMASW sem lanes round-robin in scheduled order, but each
    # lane is hardware-locked to SWDGE queue (lane % 4).  Force gather
    # instructions onto lanes consistent with their queue_num.
    if not getattr(tsa.TileClockTick, "_gather_lane_patched", False):
        _orig_assign_tick = tsa.TileClockTick._assign_tick

        def _patched_assign_tick(self, inst):
            if isinstance(inst, mybir.InstDMAGatherAnt):
                q = inst.queue_num
                tog = getattr(self, "_gather_lane_toggle", None)
                if tog is None:
                    tog = self._gather_lane_toggle = {}
                k = tog.get(q, 0)
                tog[q] = k ^ 1
                saved = self.next_sw_dma_idx
                self.next_sw_dma_idx = q + 4 * k
                try:
                    return _orig_assign_tick(self, inst)
                finally:
                    self.next_sw_dma_idx = saved
            return _orig_assign_tick(self, inst)

        tsa.TileClockTick._assign_tick = _patched_assign_tick
        tsa.TileClockTick._gather_lane_patched = True

    f32 = mybir.dt.float32
    f16 = mybir.dt.float16
    i16 = mybir.dt.int16
    AF = mybir.ActivationFunctionType
    ALU = mybir.AluOpType
    AX = mybir.AxisListType

    b_core, b_stage, n_col, nwin, caps, na = _dims()
    NB = b_stage // 128              # b-chunks per t (2 at full scale)
    NBF = b_core // 128              # full-core b-chunks (4)
    TQ = T // NQ                     # t per quarter (8)
    TPT = 2                          # t's per MLP tile
    TILE = TPT * b_stage             # columns per MLP tile (512)
    CPT = TPT * NB                   # 128-col chunks per tile (4)
    COLS_Q = b_stage * TQ            # columns per quarter (2048)
    TLQ = COLS_Q // TILE             # MLP tiles per quarter (4)
    na_slots = na // 128
    GMAX = 4096                      # max idx per gather call

    nc = bacc.Bacc("TRN2", target_bir_lowering=False, debug=debug,
                   num_swdge_queues=4)
    _qctr = [0]

    def _nextq():
        q = _qctr[0] % 4
        _qctr[0] += 1
        return q

    entp = nc.dram_tensor("entp", [N_ENTITY, 2 * DIM], f16, kind="ExternalInput")
    w1r = nc.dram_tensor("w1r", [DIM + N_RELATION, 128], f16, kind="ExternalInput")
    w2 = nc.dram_tensor("w2", [64, 64], f16, kind="ExternalInput")
    wflip = nc.dram_tensor("wflip", [128, 65], f16, kind="ExternalInput")
    items16 = nc.dram_tensor("items16", [128, max(b_core // 16, 1)], i16,
                             kind="ExternalInput")
    idxA, idxQ, ohr = {}, {}, {}
    for s in range(NS):
        idxA[s] = nc.dram_tensor(f"idxA{s}", [128, na // 16], i16,
                                 kind="ExternalInput")
        idxQ[s] = nc.dram_tensor(f"idxQ{s}", [128, 2 * n_col // 16], i16,
                                 kind="ExternalInput")
        ohr[s] = nc.dram_tensor(f"ohr{s}", [N_RELATION, n_col], f16,
                                kind="ExternalInput")
    scores_hbm = nc.dram_tensor("scores", [b_core], f32, kind="ExternalOutput")

    # flat list of (window, offset, count) phase-A calls, <=GMAX idx
    acalls = []
    off = 0
    for w in range(nwin):
        nw_ = caps[w]
        nsplit = max(1, -(-nw_ // GMAX))
        step = -(-(-(-nw_ // nsplit)) // 128) * 128
        done = 0
        while done < nw_:
            nn = min(step, nw_ - done)
            acalls.append((w, off + done, nn))
            done += nn
        off += nw_

    with tile.TileContext(nc) as tc:
        with (
            tc.tile_pool(name="const", bufs=1) as cp,
            tc.tile_pool(name="tab", bufs=2) as tabp,
            tc.tile_pool(name="xt", bufs=3) as xtp,
            tc.tile_pool(name="oh", bufs=3) as ohp,
            tc.tile_pool(name="te", bufs=2) as tep,
            tc.tile_pool(name="gsq", bufs=2) as gsp,
            tc.tile_pool(name="work", bufs=2) as wp,
            tc.tile_pool(name="idx", bufs=3) as ixp,
            tc.tile_pool(name="keep", bufs=1) as kp,
            tc.tile_pool(name="ps1", bufs=2, space="PSUM") as ps1,
            tc.tile_pool(name="ps2", bufs=2, space="PSUM") as ps2,
            tc.tile_pool(name="psF", bufs=2, space="PSUM") as psF,
            tc.tile_pool(name="psT", bufs=2, space="PSUM") as psT,
        ):
            ident = cp.tile([128, 128], f32)
            make_identity(nc, ident[:])
            ident16 = cp.tile([128, 128], f16)
            nc.vector.tensor_copy(out=ident16[:], in_=ident[:])
            w1h_sb = cp.tile([DIM, 128], f16)
            nc.sync.dma_start(out=w1h_sb[:], in_=w1r[0:DIM, :])
            r1p_sb = cp.tile([N_RELATION, 128], f16)
            nc.sync.dma_start(out=r1p_sb[:], in_=w1r[DIM:DIM + N_RELATION, :])
            w2_sb = cp.tile([64, 64], f16)
            nc.sync.dma_start(out=w2_sb[:], in_=w2[:])
            wflip_sb = cp.tile([128, 65], f16)
            nc.sync.dma_start(out=wflip_sb[:], in_=wflip[:])

            items_sb = cp.tile([128, max(b_core // 16, 1)], i16)
            nc.sync.dma_start(out=items_sb[:], in_=items16[:])
            iorig = kp.tile([128, NBF, 2 * DIM], f16)
            nc.gpsimd.dma_gather(
                out_ap=iorig[:], in_ap=entp[:], idxs_ap=items_sb[:],
                num_idxs=b_core, num_idxs_reg=b_core, elem_size=2 * DIM,
                queue_num=_nextq(), single_packet=False)

            otl = {}                 # k -> [128, NBF, DIM] f32 layer outputs
            uorig_bm = kp.tile([128, NBF, DIM], f32)
            tables = [None] * NS
            iqs = [None] * NS

            def emit_A(s):
                ia = ixp.tile([128, na // 16], i16, name=f"ia{s}", tag="ia")
                nc.sync.dma_start(out=ia[:], in_=idxA[s][:])
                iq = ixp.tile([128, 2 * n_col // 16], i16, name=f"iq{s}",
                              tag="iq")
                nc.sync.dma_start(out=iq[:], in_=idxQ[s][:])
                iqs[s] = iq
                table = tabp.tile([128, na_slots, 2 * DIM], f16,
                                  name=f"table{s}", tag="table")
                tables[s] = table
                for (w, o2, nn) in acalls:
                    nc.gpsimd.dma_gather(
                        out_ap=table[:, o2 // 128:(o2 + nn) // 128, :],
                        in_ap=entp[WIN * w: min(WIN * (w + 1), N_ENTITY), :],
                        idxs_ap=ia[:, o2 // 16:(o2 + nn) // 16],
                        num_idxs=nn, num_idxs_reg=nn, elem_size=2 * DIM,
                        queue_num=_nextq(), single_packet=False)

            def emit_body(s):
                k, half = s // 2, s % 2
                tw, l = TL_LIST[k]
                table, iq = tables[s], iqs[s]

                exh = kp.tile([128, NB, T], f16, name=f"exh{s}", tag="exh",
                              bufs=2)
                nmh = kp.tile([128, NB, NQ], f32, name=f"nmh{s}", tag="nmh",
                              bufs=2)
                esh = kp.tile([128, NB, NQ], f32, name=f"esh{s}", tag="esh",
                              bufs=2)
                psum_t = kp.tile([128, NB, NQ, DIM], f32, name=f"pst{s}",
                                 tag="pst", bufs=2)
                if tw == "u" and l == 0:
                    uacc = kp.tile([64, b_stage], f32, name=f"uacc{s}",
                                   tag="uacc", bufs=2)

                for qu in range(NQ):
                    # one-hot relations, off the gather critical chain
                    oh = ohp.tile([N_RELATION, COLS_Q], f16,
                                  name=f"oh{s}_{qu}", tag="oh")
                    nc.sync.dma_start(
                        out=oh[:], in_=ohr[s][:, qu * COLS_Q:(qu + 1) * COLS_Q])
                    xt = xtp.tile([128, 1, 2 * COLS_Q], f16,
                                  name=f"xt{s}_{qu}", tag="xt")
                    # h rows -> cols [0, COLS_Q), t -> [COLS_Q, 2C);
                    # striped over all 4 SWDGE queues for wall-time
                    GSPL = 4
                    gn = 2 * COLS_Q // GSPL
                    iqo = qu * (2 * COLS_Q) // 16
                    for j in range(GSPL):
                        nc.gpsimd.dma_gather(
                            out_ap=xt[:, :, j * gn:(j + 1) * gn],
                            in_ap=table[:],
                            idxs_ap=iq[:, iqo + j * gn // 16:
                                       iqo + (j + 1) * gn // 16],
                            num_idxs=gn, num_idxs_reg=gn,
                            elem_size=2 * DIM, transpose=True,
                            sbuf_tokens_per_rank=128,
                            sbuf_free_dim_per_rank=4 * DIM,
                            queue_num=_nextq(), single_packet=False)

                    te_sb = tep.tile([128, TQ, NB, DIM], f16,
                                     name=f"te{s}_{qu}", tag="te")
                    gsq = gsp.tile([128, TQ, NB, 65], f16,
                                   name=f"gs{s}_{qu}", tag="gsq")
                    for ti in range(TLQ):
                        cols = slice(ti * TILE, (ti + 1) * TILE)
                        tt0 = ti * TPT
                        p1 = ps1.tile([128, TILE], f32, space="PSUM",
                                      name="p1", tag="p1")
                        nc.tensor.matmul(out=p1[:],
                                         lhsT=w1h_sb[:],
                                         rhs=xt[0:DIM, 0, cols],
                                         start=True, stop=False)
                        nc.tensor.matmul(out=p1[:],
                                         lhsT=r1p_sb[:],
                                         rhs=oh[:, cols],
                                         start=False, stop=True)
                        # t rows back to batch-major via PE transpose
                        pt = psT.tile([128, CPT, DIM], f16, space="PSUM",
                                      name="pt", tag="pt")
                        for c in range(CPT):
                            c0 = COLS_Q + ti * TILE + c * 128
                            nc.tensor.transpose(
                                out=pt[:, c, :],
                                in_=xt[0:DIM, 0, c0:c0 + 128],
                                identity=ident16[0:DIM, 0:DIM])
                        nc.scalar.activation(
                            out=te_sb[:, tt0:tt0 + TPT, :, :].rearrange(
                                "p t q d -> p (t q) d"),
                            in_=pt[:], func=AF.Copy)
                        lflip = wp.tile([128, TILE], f16, name="lflip",
                                        tag="lflip")
                        nc.vector.tensor_scalar(
                            out=lflip[0:64, :], in0=p1[0:64, :], scalar1=0.0,
                            scalar2=None, op0=ALU.max)
                        r1s = wp.tile([64, TILE], f16, name="r1s", tag="r1s")
                        nc.scalar.activation(out=r1s[:], in_=p1[64:128, :],
                                             func=AF.Relu)
                        p2 = ps2.tile([64, TILE], f32, space="PSUM",
                                      name="p2", tag="p2")
                        nc.tensor.matmul(out=p2[:], lhsT=w2_sb[:], rhs=r1s[:],
                                         start=True, stop=True)
                        nc.vector.tensor_scalar(
                            out=lflip[64:128, :], in0=p2[:], scalar1=0.0,
                            scalar2=None, op0=ALU.max)
                        pf = psF.tile([128, CPT, 65], f32, space="PSUM",
                                      name="pf", tag="pf")
                        for c in range(CPT):
                            nc.tensor.matmul(out=pf[:, c, :],
                                             lhsT=lflip[:, c * 128:(c + 1) * 128],
                                             rhs=wflip_sb[:],
                                             start=True, stop=True)
                        nc.scalar.activation(
                            out=gsq[:, tt0:tt0 + TPT, :, :].rearrange(
                                "p t q d -> p (t q) d"),
                            in_=pf[:], func=AF.Sigmoid)

                    # ---- per-quarter softmax partials + weighted sums ----
                    nc.vector.tensor_reduce(
                        out=nmh[:, :, qu:qu + 1],
                        in_=gsq[:, :, :, 64].rearrange("p t q -> p q t"),
                        axis=AX.X, op=ALU.max, negate=True)
                    for q in range(NB):
                        nc.scalar.activation(
                            out=exh[:, q, qu * TQ:(qu + 1) * TQ],
                            in_=gsq[:, :, q, 64:65], func=AF.Exp,
                            bias=nmh[:, q, qu:qu + 1],
                            accum_out=esh[:, q, qu:qu + 1])
                        wmul = wp.tile([128, TQ, DIM], f16, name="wmul",
                                       tag="wmul")
                        nc.vector.tensor_tensor(
                            out=wmul[:], in0=te_sb[:, :, q, :],
                            in1=exh[:, q, qu * TQ:(qu + 1) * TQ, None]
                                .to_broadcast([128, TQ, DIM]),
                            op=ALU.mult)
                        pmul = wp.tile([128, TQ, DIM], f16, name="pmul",
                                       tag="pmul")
                        nc.vector.tensor_tensor(
                            out=pmul[:], in0=wmul[:],
                            in1=gsq[:, :, q, 0:64], op=ALU.mult)
                        nc.vector.tensor_reduce(
                            out=psum_t[:, q, qu, :],
                            in_=pmul[:].rearrange("p t d -> p d t"),
                            axis=AX.X, op=ALU.add)

                    if tw == "u" and l == 0:
                        ured = wp.tile([64, b_stage], f32, name="ured",
                                       tag="ured")
                        nc.vector.tensor_reduce(
                            out=ured[:],
                            in_=xt[0:DIM, 0, 0:COLS_Q]
                                .rearrange("p (t b) -> p b t", b=b_stage),
                            axis=AX.X, op=ALU.add)
                        if qu == 0:
                            nc.vector.tensor_copy(out=uacc[:], in_=ured[:])
                        else:
                            nc.vector.tensor_add(out=uacc[:], in0=uacc[:],
                                                 in1=ured[:])

                # ---- combine quarters with softmax renormalization ----
                nmall = wp.tile([128, NB, 1], f32, name="nmall", tag="nmall")
                nc.vector.tensor_reduce(out=nmall[:], in_=nmh[:],
                                        axis=AX.X, op=ALU.min)
                dif = wp.tile([128, NB, NQ], f32, name="dif", tag="dif")
                nc.vector.tensor_tensor(
                    out=dif[:], in0=nmall[:].to_broadcast([128, NB, NQ]),
                    in1=nmh[:], op=ALU.subtract)
                sc = wp.tile([128, NB, NQ], f32, name="sc", tag="sc")
                nc.scalar.activation(out=sc[:], in_=dif[:], func=AF.Exp)
                stmp = wp.tile([128, NB, NQ], f32, name="stmp", tag="stmp")
                nc.vector.tensor_tensor(out=stmp[:], in0=esh[:], in1=sc[:],
                                        op=ALU.mult)
                tot = wp.tile([128, NB, 1], f32, name="tot", tag="tot")
                nc.vector.tensor_reduce(out=tot[:], in_=stmp[:], axis=AX.X,
                                        op=ALU.add)
                rec = wp.tile([128, NB, 1], f32, name="rec", tag="rec")
                nc.vector.reciprocal(out=rec[:], in_=tot[:])
                pw = wp.tile([128, NB, NQ, DIM], f32, name="pw", tag="pw")
                nc.vector.tensor_tensor(
                    out=pw[:], in0=psum_t[:],
                    in1=sc[:, :, :, None].to_broadcast([128, NB, NQ, DIM]),
                    op=ALU.mult)
                osum = wp.tile([128, NB, DIM], f32, name="osum", tag="osum")
                nc.vector.tensor_reduce(
                    out=osum[:], in_=pw[:].rearrange("p q h d -> p q d h"),
                    axis=AX.X, op=ALU.add)
                if k not in otl:
                    otl[k] = kp.tile([128, NBF, DIM], f32, name=f"otl{k}",
                                     tag=f"otl{k}")
                nc.vector.tensor_tensor(
                    out=otl[k][:, half * NB:(half + 1) * NB, :], in0=osum[:],
                    in1=rec[:].to_broadcast([128, NB, DIM]), op=ALU.mult)

                if tw == "u" and l == 0:
                    put = psT.tile([128, NB, DIM], f32, space="PSUM",
                                   name="put", tag="pt")
                    for q in range(NB):
                        nc.tensor.transpose(
                            out=put[:, q, :],
                            in_=uacc[:, q * 128:(q + 1) * 128],
                            identity=ident[0:64, 0:64])
                    nc.scalar.activation(
                        out=uorig_bm[:, half * NB:(half + 1) * NB, :],
                        in_=put[:], func=AF.Copy)

            # ---- software-pipelined emission ----
            for s in range(NS + 1):
                if s < NS:
                    emit_A(s)
                if s >= 1:
                    emit_body(s - 1)

            # ---- scores (2.0 gate scale twice, 1/T origin mean) ----
            m = wp.tile([128, NBF, DIM], f32, name="m", tag="m")
            nc.vector.tensor_tensor(out=m[:], in0=uorig_bm[:],
                                    in1=iorig[:, :, 0:DIM], op=ALU.mult)
            acc = wp.tile([128, NBF, DIM], f32, name="macc", tag="macc")
            nc.vector.tensor_scalar(out=acc[:], in0=m[:], scalar1=1.0 / T,
                                    scalar2=None, op0=ALU.mult)
            for ku, ki in ((0, 2), (1, 3)):
                mu = wp.tile([128, NBF, DIM], f32, name="mu", tag="mu")
                nc.vector.tensor_tensor(out=mu[:], in0=otl[ku][:],
                                        in1=otl[ki][:], op=ALU.mult)
                nc.vector.tensor_scalar(out=mu[:], in0=mu[:], scalar1=4.0,
                                        scalar2=None, op0=ALU.mult)
                nc.vector.tensor_add(out=acc[:], in0=acc[:], in1=mu[:])
            ssum = wp.tile([128, NBF, 1], f32, name="ssum", tag="ssum")
            nc.vector.tensor_reduce(out=ssum[:], in_=acc[:], axis=AX.X,
                                    op=ALU.add)
            sc_all = kp.tile([128, NBF], f32)
            nc.scalar.activation(out=sc_all[:], in_=ssum[:, :, 0],
                                 func=AF.Sigmoid)
            nc.sync.dma_start(out=scores_hbm.rearrange("(s p) -> p s", p=128),
                              in_=sc_all[:])
    nc.compile()
    return nc


def _make_in_maps(inputs):
    b_core, b_stage, n_col, nwin, caps, na = _dims()
    ent = np.asarray(inputs["ent_emb"], np.float32)
    rel = np.asarray(inputs["rel_emb"], np.float32)
    att_w1 = np.asarray(inputs["att_w1"], np.float32)
    att_w2 = np.asarray(inputs["att_w2"], np.float32)
    att_w3 = np.asarray(inputs["att_w3"], np.float32)
    gate_w1 = np.asarray(inputs["gate_w1"], np.float32)
    gate_w2 = np.asarray(inputs["gate_w2"], np.float32)
    items = np.asarray(inputs["items"]).astype(np.int64)
    idx6 = {n: np.asarray(inputs[n]).astype(np.int64)
            for n in ("user_h", "user_r", "user_t", "item_h", "item_r",
                      "item_t")}

    entp = np.zeros((N_ENTITY, 2 * DIM), np.float16)
    entp[:, 0:DIM] = ent.astype(np.float16)
    w1h = np.concatenate([gate_w1[:DIM], att_w1[:DIM]], axis=1)
    r1p = rel @ np.concatenate([gate_w1[DIM:], att_w1[DIM:]], axis=1)
    w1r = np.concatenate([w1h, r1p], axis=0).astype(np.float16)
    wflip = np.zeros((128, 65), np.float16)
    wflip[0:64, 0:64] = gate_w2.astype(np.float16)
    wflip[64:128, 64:65] = att_w3.astype(np.float16)

    in_maps = []
    for c in range(N_CORES):
        im = {
            "entp": entp, "w1r": w1r, "w2": att_w2.astype(np.float16),
            "wflip": wflip,
            "items16": _wrap_idx16(
                items[c * b_core:(c + 1) * b_core].astype(np.int16)),
        }
        for s in range(NS):
            k, half = s // 2, s % 2
            tw, l = TL_LIST[k]
            pre = "user" if tw == "u" else "item"
            lo = c * b_core + half * b_stage
            sl = slice(lo, lo + b_stage)
            h = idx6[f"{pre}_h"][l, sl].T.ravel()
            t = idx6[f"{pre}_t"][l, sl].T.ravel()
            r = idx6[f"{pre}_r"][l, sl].T.ravel()
            ia, hl, tl_ = _host_prep_tl(h, t)
            im[f"idxA{s}"] = _wrap_idx16(ia)
            # per-quarter [h_cols ++ t_cols] index stream
            cq = n_col // NQ
            parts = []
            for qu in range(NQ):
                parts.append(_wrap_idx16(hl[qu * cq:(qu + 1) * cq]))
                parts.append(_wrap_idx16(tl_[qu * cq:(qu + 1) * cq]))
            im[f"idxQ{s}"] = np.concatenate(parts, axis=1)
            oh = (r[None, :] == np.arange(N_RELATION)[:, None])
            im[f"ohr{s}"] = oh.astype(np.float16)
        in_maps.append(im)
    return in_maps


def kernel(**inputs):
    global _NC_CACHE
    import os
    from concourse.bass_utils import run_bass_kernel_spmd

    if _NC_CACHE is None:
        _NC_CACHE = _build_program()
    nc = _NC_CACHE
    in_maps = _make_in_maps(inputs)
    trace = bool(int(os.environ.get("CKAN_TRACE", "0")))
    res = run_bass_kernel_spmd(nc, in_maps, core_ids=list(range(N_CORES)),
                               trace=trace)
    if trace and res.exec_time_ns is not None:
        print(f"HW exec time: {res.exec_time_ns} ns")
    if trace and res.instructions_and_trace is not None:
        print(f"trace path: {res.instructions_and_trace[1]}")
    b_core = B // N_CORES
    out = np.concatenate([res.results[c]["scores"] for c in range(N_CORES)])
    return out.astype(np.float32)
